# revision 6
# baseline (speedup 1.0000x reference)
"""Sparse-attention Trainium2 kernel (8 NeuronCores, sequence-parallel).

Problem (hardcoded): B=1, S=4096, H=1024, NH=16, D=64, K=32.

Sharding: fully sequence-parallel. Core c owns query rows [512c, 512c+512).
It computes q/k/v for its own rows against the FULL weight matrices (1/8 of
the total FLOPs, no communication), publishes its k|v rows via four per-st
chunked 8-way AllGathers (bf16, pipelined with the rest of phase A so the
exchange overlaps the q projection), then dma_gathers per-query k/v rows for
ALL 16 heads at once (4 KB/descriptor), computes the sparse attention for its
rows and the o-projection. Host concatenates row slices + bo.

kv_full row order is permuted for the chunked AllGather: global row
g = 512c + 128st + i lands at st*1024 + c*128 + i; the host remaps the
gather indices accordingly.

Per-tile layout (16 queries/tile, 32 tiles/core): gather position
pos = j*16 + q -> SBUF partition p = 16*(j%8) + q, chunk cc = j//8, so
partition p holds query tb+p%16 and slot block b = p//16 (slot j = 8cc+b).
Cross-partition sums over the 8 slot blocks (softmax denominator, AV
accumulation) are TensorE matmuls against a static 0/1 selection matrix
S16[p, m] = (p%16 == m). q is replicated to 128 partitions by a tiny
128-descriptor dma_gather from q in DRAM.

DVE diet (the attention phase is Vector-bound): the d-reduction for the
logits is a pure tensor_tensor halving tree (tensor_reduce only has a 1x
uop; TT adds hit the 2x bf16 mode), exp writes the pair-expanded e2 tile
directly on ScalarE, and the softmax normalization is applied to e (one
[128, 4, 16, 2] mul against a PE-broadcast 1/den) instead of to the
[16, 1024] AV output, so the PSUM evacuation is a plain ScalarE copy.
Broadcast multiplies pair-expand the scalar operand so the VectorE runs
in its 2x bf16 mode; kvsel gathers use single_packet=False (large
single-packet SWDGE gathers hard-fault). Attention-side elementwise runs
on VectorE only; GpSimd does only early-bound work (gathers) -- putting
softmax-dependent ops on its in-order queue stalls future tiles' gathers.
"""

import os
from contextlib import ExitStack

import numpy as np
import ml_dtypes

S, H, NH, D, K = 4096, 1024, 16, 64, 32
NCORES = 8
SC = S // NCORES            # 512 rows per core
QT = 16                     # queries per attention tile
NTB = SC // QT              # 32 attention tiles per core
NST = SC // 128             # 4 projection s-tiles per core
CH = NH * D                 # 1024 kv channels per tensor
ROW = 2 * CH                # 2048 bf16 elems per kv row (4 KB)
NCC = K // 8                # 4 slot chunks per tile
BF16 = ml_dtypes.bfloat16

_nc_cache = None


def build_nc(mode="full"):
    import concourse.bass as bass
    import concourse.mybir as mybir
    import concourse.tile as tile
    from concourse import bacc
    from concourse.tile_rust import add_dep_helper
    from concourse.bass import ts, ds

    dt = mybir.dt
    nc = bacc.Bacc("TRN2", target_bir_lowering=False, debug=False,
                   num_devices=NCORES)

    xT = nc.dram_tensor("xT", [H, SC], dt.bfloat16, kind="ExternalInput")
    w3T = nc.dram_tensor("w3T", [H, 3 * CH], dt.bfloat16, kind="ExternalInput")
    woT = nc.dram_tensor("woT", [CH, H], dt.bfloat16, kind="ExternalInput")
    gb = nc.dram_tensor("gb", [128, NTB * 4 * NH], dt.float32, kind="ExternalInput")
    idx16 = nc.dram_tensor("idx16", [128, NTB * (QT * K // 16)], dt.int16,
                           kind="ExternalInput")
    s16d = nc.dram_tensor("s16", [128, 16], dt.bfloat16, kind="ExternalInput")
    s16td = nc.dram_tensor("s16t", [16, 128], dt.float32, kind="ExternalInput")
    qidxd = nc.dram_tensor("qidx16", [128, NTB * 8], dt.int16, kind="ExternalInput")
    identd = nc.dram_tensor("ident", [16, 16], dt.bfloat16, kind="ExternalInput")
    outd = nc.dram_tensor("out", [SC, H], dt.float32, kind="ExternalOutput")
    kv_loc = nc.dram_tensor("kv_loc", [SC, ROW], dt.bfloat16, kind="Internal")
    q_dram = nc.dram_tensor("q_dram", [SC, CH], dt.bfloat16, kind="Internal")
    kv_full = nc.dram_tensor("kv_full", [S, ROW], dt.bfloat16, kind="Internal",
                             addr_space="Shared")

    EXP = mybir.ActivationFunctionType.Exp
    COPY = mybir.ActivationFunctionType.Copy

    with ExitStack() as ctx:
        tc = ctx.enter_context(tile.TileContext(nc))
        const = ctx.enter_context(tc.tile_pool(name="const", bufs=1))

        kv_pool = ctx.enter_context(tc.tile_pool(name="kvout", bufs=2))
        ps_big = ctx.enter_context(tc.tile_pool(name="ps_big", bufs=3, space="PSUM"))
        ps_sm = ctx.enter_context(tc.tile_pool(name="ps_sm", bufs=2, space="PSUM"))

        # ---- phase-A weights first: the kv matmuls gate the collective ----
        wa = tc.tile_pool(name="wa", bufs=1)
        wap = wa.__enter__()
        xT_sb = wap.tile([128, 8, SC], dt.bfloat16)           # 1 MB, phase A only
        for kc in range(8):
            nc.sync.dma_start(xT_sb[:, kc, :], xT[ts(kc, 128), :])
        w3_sb = wap.tile([128, 8, 3 * CH], dt.bfloat16)       # 6 MB, phase A only
        for kc in range(8):
            nc.sync.dma_start(w3_sb[:, kc, ds(CH, 2 * CH)],
                              w3T[ts(kc, 128), ds(CH, 2 * CH)])
        for kc in range(8):
            nc.sync.dma_start(w3_sb[:, kc, ds(0, CH)], w3T[ts(kc, 128), ds(0, CH)])

        # ---- small resident tensors ----
        idx_sb = const.tile([128, NTB * 32], dt.int16)        # 0.25 MB
        nc.sync.dma_start(idx_sb[:], idx16[:, :])
        gb_sb = const.tile([128, NTB, 4 * NH], dt.float32)    # 1 MB
        nc.sync.dma_start(gb_sb[:], gb[:, :])
        s16_sb = const.tile([128, 16], dt.bfloat16)
        nc.sync.dma_start(s16_sb[:], s16d[:, :])
        s16t_sb = const.tile([16, 128], dt.float32)
        nc.sync.dma_start(s16t_sb[:], s16td[:, :])
        qidx_sb = const.tile([128, NTB * 8], dt.int16)
        nc.sync.dma_start(qidx_sb[:], qidxd[:, :])
        ident_sb = const.tile([16, 16], dt.bfloat16)
        nc.sync.dma_start(ident_sb[:], identd[:, :])
        wo_sb = const.tile([128, 8, H], dt.bfloat16)          # 2 MB
        for chn in range(8):
            nc.sync.dma_start(wo_sb[:, chn, :], woT[ts(chn, 128), :])

        # ---- phase A: k/v per st, chunked AllGather starts per chunk ----
        cc_insts = []
        q_stores = []
        for st in range(NST):
            kvt_cur = None
            stn = None
            for pj in (1, 2):         # k then v
                ps = ps_big.tile([128, CH], dt.float32, tag="psb")
                for n in range(2):
                    for kc in range(8):
                        nc.tensor.matmul(
                            ps[:, ts(n, 512)],
                            xT_sb[:, kc, ts(st, 128)],
                            w3_sb[:, kc, ds(pj * CH + n * 512, 512)],
                            start=(kc == 0), stop=(kc == 7))
                if pj == 1:
                    kvt_cur = kv_pool.tile([128, 2, CH], dt.bfloat16, tag="kvt")
                    nc.scalar.copy(kvt_cur[:, 0, :], ps[:])
                else:
                    nc.scalar.copy(kvt_cur[:, 1, :], ps[:])
                    stn = nc.sync.dma_start(
                        kv_loc[ts(st, 128), :],
                        kvt_cur[:].rearrange("p a b -> p (a b)"))
            cc_i = nc.gpsimd.collective_compute(
                "AllGather", mybir.AluOpType.bypass,
                replica_groups=[list(range(NCORES))],
                ins=[kv_loc[ts(st, 128), :]],
                outs=[kv_full[ts(st, 1024), :]])
            add_dep_helper(cc_i.ins, stn.ins, sync=True, reason="cc after kv store")
            cc_insts.append(cc_i)
        for st in range(NST):         # q after all k/v (overlaps AllGather)
            ps = ps_big.tile([128, CH], dt.float32, tag="psb")
            for n in range(2):
                for kc in range(8):
                    nc.tensor.matmul(
                        ps[:, ts(n, 512)],
                        xT_sb[:, kc, ts(st, 128)],
                        w3_sb[:, kc, ds(n * 512, 512)],
                        start=(kc == 0), stop=(kc == 7))
            qt = kv_pool.tile([128, CH], dt.bfloat16, tag="qt")
            nc.scalar.copy(qt[:], ps[:])
            q_stores.append(nc.sync.dma_start(q_dram[ts(st, 128), :], qt[:]))
        wa.__exit__(None, None, None)
        gat = ctx.enter_context(tc.tile_pool(name="gat", bufs=3))
        big = ctx.enter_context(tc.tile_pool(name="big", bufs=2))
        small = ctx.enter_context(tc.tile_pool(name="small", bufs=4))
        atg_pool = ctx.enter_context(tc.tile_pool(name="atg", bufs=2))
        outp = ctx.enter_context(tc.tile_pool(name="outp", bufs=2))

        # ---- phase B: per-tile sparse attention ----
        atg_cur = None
        for t in range(NTB if mode != "proj" else 0):
            st, g16 = t // 8, t % 8
            # 1. gather k/v rows: [128, NCC, ROW]
            kvsel = gat.tile([128, NCC, ROW], dt.bfloat16, tag="kvsel")
            g = nc.gpsimd.dma_gather(
                out_ap=kvsel[:], in_ap=kv_full[:, :],
                idxs_ap=idx_sb[:, ds(t * 32, 32)],
                num_idxs=QT * K, num_idxs_reg=QT * K,
                elem_size=ROW, single_packet=False)
            for cc_i in cc_insts:
                add_dep_helper(g.ins, cc_i.ins, sync=True, reason="gather after cc")

            # 2. q replicated to 128 partitions (q[p%16]) via tiny row-gather
            qrep = small.tile([128, 1, CH], dt.bfloat16, tag="qrep")
            gq = nc.gpsimd.dma_gather(
                out_ap=qrep[:], in_ap=q_dram[:, :],
                idxs_ap=qidx_sb[:, ds(t * 8, 8)],
                num_idxs=128, num_idxs_reg=128,
                elem_size=CH, single_packet=False)
            add_dep_helper(gq.ins, q_stores[st].ins, sync=True,
                           reason="qrep gather after q store")

            # 3. t1 = qrep (bcast over chunks) * k_sel
            t1 = big.tile([128, NCC, CH], dt.bfloat16, tag="t1")
            k_ap = kvsel[:, :, 0:CH]
            k_ap2, q_ap2 = bass.broadcast_tensor_aps(k_ap, qrep[:, 0:1, :])
            nc.vector.tensor_mul(t1[:], k_ap2, q_ap2)

            # 4. logits[p, (cc,h)] = sum_d t1 -- pure-TT halving tree (2x mode;
            # tensor_reduce only has a 1x uop)
            th = small.tile([128, 4 * NH, 32], dt.bfloat16, tag="th")
            t1v = t1[:].rearrange("p c (h d) -> p (c h) d", d=D)
            nc.vector.tensor_add(th[:], t1v[:, :, 0:32], t1v[:, :, 32:64])
            nc.vector.tensor_add(th[:, :, 0:16], th[:, :, 0:16], th[:, :, 16:32])
            nc.vector.tensor_add(th[:, :, 0:8], th[:, :, 0:8], th[:, :, 8:16])
            nc.vector.tensor_add(th[:, :, 0:4], th[:, :, 0:4], th[:, :, 4:8])
            nc.vector.tensor_add(th[:, :, 0:2], th[:, :, 0:2], th[:, :, 2:4])
            lgt = small.tile([128, 4 * NH], dt.float32, tag="lgt")
            lgtv = lgt[:].rearrange("p (g o) -> p g o", o=1)
            nc.vector.tensor_add(lgtv, th[:, :, 0:1], th[:, :, 1:2])
            nc.vector.tensor_add(lgt[:], lgt[:], gb_sb[:, t, :])

            # 5. e2 = exp(logits), written pair-expanded for the 2x bcast muls
            e2 = small.tile([128, NCC, NH, 2], dt.bfloat16, tag="e2")
            lgt4 = lgt[:].rearrange("p (c h o) -> p c h o", c=NCC, o=1)
            e2a, lgt4b = bass.broadcast_tensor_aps(e2[:], lgt4)
            nc.scalar.activation(e2a, lgt4b, EXP)

            # 6. 1/den broadcast: den[q, (h, w)] = sum_{b,cc} e  via S16 matmul
            # (both pair lanes carried through -> the result is already
            # pair-expanded), reciprocal, then replicated to 128 partitions
            # via an S16^T matmul
            psd = ps_sm.tile([16, 2 * NH], dt.float32, tag="pss")
            for cc in range(NCC):
                nc.tensor.matmul(psd[:],
                                 s16_sb[:],
                                 e2[:, cc, :, :].rearrange("p h w -> p (h w)"),
                                 start=(cc == 0), stop=(cc == NCC - 1))
            r16 = small.tile([16, 2 * NH], dt.float32, tag="r16")
            nc.vector.reciprocal(r16[:], psd[:])
            psr = ps_sm.tile([128, 2 * NH], dt.float32, tag="pss")
            nc.tensor.matmul(psr[:], s16t_sb[:], r16[:], start=True, stop=True)
            rbc = small.tile([128, 1, NH, 2], dt.bfloat16, tag="rbc")
            nc.scalar.copy(rbc[:].rearrange("p c h w -> p (c h w)"), psr[:])

            # 7. pre-normalize: a = e2 * (1/den)  [128, NCC, NH, 2]
            e2n = small.tile([128, NCC, NH, 2], dt.bfloat16, tag="e2n")
            e2b, rbcb = bass.broadcast_tensor_aps(e2[:], rbc[:])
            nc.vector.tensor_mul(e2n[:], e2b, rbcb)

            # 8. W = v_sel * a (bcast over d, pair-expanded so DVE hits 2x)
            W = big.tile([128, NCC, CH], dt.bfloat16, tag="W")
            v_ap2, e_ap2 = bass.broadcast_tensor_aps(
                kvsel[:, :, CH:ROW].rearrange("p c (h dd w) -> p c h dd w", dd=32, w=2),
                e2n[:].rearrange("p c h (dd w) -> p c h dd w", dd=1, w=2))
            nc.vector.tensor_mul(
                W[:].rearrange("p c (h dd w) -> p c h dd w", dd=32, w=2),
                v_ap2, e_ap2)

            # 9. A[q, hd] = sum_{b,cc} W  via S16 matmul (PSUM accumulate)
            psA = ps_big.tile([16, CH], dt.float32, tag="psb")
            for n in range(2):
                for cc in range(NCC):
                    nc.tensor.matmul(psA[:, ts(n, 512)], s16_sb[:],
                                     W[:, cc, ts(n, 512)],
                                     start=(cc == 0), stop=(cc == NCC - 1))
            A_sb = small.tile([16, CH], dt.bfloat16, tag="A_sb")
            nc.scalar.copy(A_sb[:], psA[:])

            # 10. A^T chunks via PE transpose -> group buffer [128, 8, 128]
            if g16 == 0:
                atg_cur = atg_pool.tile([128, 8, 128], dt.bfloat16, tag="atg")
            psT = ps_sm.tile([128, 8, QT], dt.bfloat16, tag="pss")
            for chk in range(8):
                nc.tensor.transpose(psT[:, chk, :], A_sb[:, ts(chk, 128)],
                                    ident_sb[:])
            nc.scalar.copy(atg_cur[:, :, ds(QT * g16, QT)], psT[:])

            # 11. o-proj per group of 8 tiles (128 query rows)
            if g16 == 7:
                psP = ps_big.tile([128, H], dt.float32, tag="psb")
                for n in range(2):
                    for chk in range(8):
                        nc.tensor.matmul(psP[:, ts(n, 512)], atg_cur[:, chk, :],
                                         wo_sb[:, chk, ts(n, 512)],
                                         start=(chk == 0), stop=(chk == 7))
                ot = outp.tile([128, H], dt.float32, tag="ot")
                nc.scalar.copy(ot[:], psP[:])
                nc.sync.dma_start(outd[ts(st, 128), :], ot[:])

    nc.compile()
    return nc


def prep_inputs(x, idx, valid, geo_bias, Wq, Wk, Wv, Wo, bo):
    """Host-side shard prep. Returns (in_maps, bo_f32)."""
    x = np.asarray(x)
    idx = np.asarray(idx)
    geo_bias = np.asarray(geo_bias)
    Wq, Wk, Wv, Wo = (np.asarray(w) for w in (Wq, Wk, Wv, Wo))
    bo = np.asarray(bo, dtype=np.float32)

    x2 = x.reshape(S, H)
    scale = np.float32(1.0 / np.sqrt(D))
    w3T = np.ascontiguousarray(
        np.concatenate([(Wq * scale).T, Wk.T, Wv.T], axis=1).astype(BF16))
    woT = np.ascontiguousarray(Wo.T.astype(BF16))
    s16 = np.zeros((128, 16), dtype=BF16)
    s16[np.arange(128), np.arange(128) % 16] = 1
    s16t = np.ascontiguousarray(s16.T.astype(np.float32))
    ident = np.eye(16, dtype=BF16)
    # qrep gather: tile t, pos p -> q row t*16 + p%16
    qidx = np.empty((16, NTB * 8), dtype=np.int16)
    for t in range(NTB):
        lin = (t * QT + np.arange(128) % 16).astype(np.int16)
        qidx[:, t * 8:(t + 1) * 8] = lin.reshape(8, 16).T
    qidx = np.ascontiguousarray(np.tile(qidx, (8, 1)))

    # kv_full row permutation for the chunked AllGather:
    # global row g = 512c + 128st + i  ->  st*1024 + c*128 + i
    g = np.arange(S)
    remap = ((g % 512) // 128) * 1024 + (g // 512) * 128 + (g % 128)
    remap = remap.astype(np.int64)

    in_maps = []
    for c in range(NCORES):
        rb = c * SC
        xTc = np.ascontiguousarray(x2[rb:rb + SC].T.astype(BF16))

        # gather indices: tile t, pos = j*16 + q -> remap[idx[rb + t*16 + q, j]]
        idxc = np.empty((16, NTB * 32), dtype=np.int16)
        for t in range(NTB):
            blk = remap[idx[rb + t * QT: rb + (t + 1) * QT, :]]  # [16 q, 32 j]
            lin = blk.T.reshape(-1)                              # pos = j*16+q
            idxc[:, t * 32:(t + 1) * 32] = lin.reshape(32, 16).T.astype(np.int16)
        idxc = np.ascontiguousarray(np.tile(idxc, (8, 1)))

        # geo bias: gb[p=(b,qq), t, cc*16+h] = geo_bias[h, rb+t*16+qq, cc*8+b]
        gg = geo_bias[:, rb:rb + SC, :]                        # [h, 512, j]
        g2 = gg.reshape(NH, NTB, QT, NCC, 8)                   # [h, t, qq, cc, b]
        gbt = g2.transpose(4, 2, 1, 3, 0).reshape(128, NTB * 4 * NH)
        gbt = np.ascontiguousarray(gbt, dtype=np.float32)

        in_maps.append({
            "xT": xTc,
            "w3T": w3T,
            "woT": woT,
            "gb": gbt,
            "idx16": idxc,
            "s16": s16,
            "s16t": s16t,
            "qidx16": qidx,
            "ident": ident,
        })
    return in_maps, bo


def kernel(x, idx, valid, geo_bias, Wq, Wk, Wv, Wo, bo):
    global _nc_cache
    from concourse.bass_utils import run_bass_kernel_spmd

    if _nc_cache is None:
        _nc_cache = build_nc()
    nc = _nc_cache

    in_maps, bo_f32 = prep_inputs(x, idx, valid, geo_bias, Wq, Wk, Wv, Wo, bo)
    res = run_bass_kernel_spmd(nc, in_maps, core_ids=list(range(NCORES)),
                               trace=bool(int(os.environ.get("KTRACE", "0"))))
    out = np.concatenate([r["out"] for r in res.results], axis=0)
    out = out + bo_f32[None, :]
    if res.exec_time_ns is not None:
        kernel.last_exec_time_ns = res.exec_time_ns
    kernel.last_results = res
    return out.reshape(1, S, H).astype(np.float32)


# revision 9
# speedup vs baseline: 1.0890x; 1.0890x over previous
"""Sparse-attention Trainium2 kernel (8 NeuronCores, sequence-parallel).

Problem (hardcoded): B=1, S=4096, H=1024, NH=16, D=64, K=32.

Sharding: fully sequence-parallel. Core c owns query rows [512c, 512c+512).
It computes q/k/v for its own rows against the FULL weight matrices (1/8 of
the total FLOPs, no communication), publishes its k|v rows via an
8-way AllGather (bf16; triggered as soon as the kv stores land, well before
the q projection finishes), then dma_gathers per-query k/v rows for
ALL 16 heads at once (4 KB/descriptor), computes the sparse attention for its
rows and the o-projection. Host concatenates row slices + bo.

Per-tile layout (16 queries/tile, 32 tiles/core): gather position
pos = j*16 + q -> SBUF partition p = 16*(j%8) + q, chunk cc = j//8, so
partition p holds query tb+p%16 and slot block b = p//16 (slot j = 8cc+b).
Cross-partition sums over the 8 slot blocks (softmax denominator, AV
accumulation) are TensorE matmuls against a static 0/1 selection matrix
S16[p, m] = (p%16 == m). q is replicated to 128 partitions by a tiny
128-descriptor dma_gather from q in DRAM.

DVE diet (the attention phase is Vector-bound): the d-reduction for the
logits is a pure tensor_tensor halving tree (tensor_reduce only has a 1x
uop; TT adds hit the 2x bf16 mode), exp writes the pair-expanded e2 tile
directly on ScalarE, and the softmax normalization is applied to e (one
[128, 4, 16, 2] mul against a PE-broadcast 1/den) instead of to the
[16, 1024] AV output, so the PSUM evacuation is a plain ScalarE copy.
Broadcast multiplies pair-expand the scalar operand so the VectorE runs
in its 2x bf16 mode; kvsel gathers use single_packet=False (large
single-packet SWDGE gathers hard-fault). Attention-side elementwise runs
on VectorE only; GpSimd does only early-bound work (gathers) -- putting
softmax-dependent ops on its in-order queue stalls future tiles' gathers.
"""

import os
from contextlib import ExitStack

import numpy as np
import ml_dtypes

S, H, NH, D, K = 4096, 1024, 16, 64, 32
NCORES = 8
SC = S // NCORES            # 512 rows per core
QT = 16                     # queries per attention tile
NTB = SC // QT              # 32 attention tiles per core
NST = SC // 128             # 4 projection s-tiles per core
CH = NH * D                 # 1024 kv channels per tensor
ROW = 2 * CH                # 2048 bf16 elems per kv row (4 KB)
NCC = K // 8                # 4 slot chunks per tile
BF16 = ml_dtypes.bfloat16

_nc_cache = None


def build_nc(mode="full"):
    import concourse.bass as bass
    import concourse.mybir as mybir
    import concourse.tile as tile
    from concourse import bacc
    from concourse.tile_rust import add_dep_helper
    from concourse.bass import ts, ds

    dt = mybir.dt
    nc = bacc.Bacc("TRN2", target_bir_lowering=False, debug=False,
                   num_devices=NCORES)

    xT = nc.dram_tensor("xT", [H, SC], dt.bfloat16, kind="ExternalInput")
    w3T = nc.dram_tensor("w3T", [H, 3 * CH], dt.bfloat16, kind="ExternalInput")
    woT = nc.dram_tensor("woT", [CH, H], dt.bfloat16, kind="ExternalInput")
    gb = nc.dram_tensor("gb", [128, NTB * 4 * NH], dt.float32, kind="ExternalInput")
    idx16 = nc.dram_tensor("idx16", [128, NTB * (QT * K // 16)], dt.int16,
                           kind="ExternalInput")
    s16d = nc.dram_tensor("s16", [128, 16], dt.bfloat16, kind="ExternalInput")
    qidxd = nc.dram_tensor("qidx16", [128, NTB * 8], dt.int16, kind="ExternalInput")
    identd = nc.dram_tensor("ident", [16, 16], dt.bfloat16, kind="ExternalInput")
    outd = nc.dram_tensor("out", [SC, H], dt.float32, kind="ExternalOutput")
    kv_loc = nc.dram_tensor("kv_loc", [SC, ROW], dt.bfloat16, kind="Internal")
    q_dram = nc.dram_tensor("q_dram", [SC, CH], dt.bfloat16, kind="Internal")
    kv_full = nc.dram_tensor("kv_full", [S, ROW], dt.bfloat16, kind="Internal",
                             addr_space="Shared")

    EXP = mybir.ActivationFunctionType.Exp
    COPY = mybir.ActivationFunctionType.Copy

    with ExitStack() as ctx:
        tc = ctx.enter_context(tile.TileContext(nc))
        const = ctx.enter_context(tc.tile_pool(name="const", bufs=1))

        kv_pool = ctx.enter_context(tc.tile_pool(name="kvout", bufs=2))
        ps_big = ctx.enter_context(tc.tile_pool(name="ps_big", bufs=3, space="PSUM"))
        ps_sm = ctx.enter_context(tc.tile_pool(name="ps_sm", bufs=2, space="PSUM"))

        # ---- phase-A weights first: the kv matmuls gate the collective ----
        wa = tc.tile_pool(name="wa", bufs=1)
        wap = wa.__enter__()
        xT_sb = wap.tile([128, 8, SC], dt.bfloat16)           # 1 MB, phase A only
        for kc in range(8):
            nc.sync.dma_start(xT_sb[:, kc, :], xT[ts(kc, 128), :])
        w3_sb = wap.tile([128, 8, 3 * CH], dt.bfloat16)       # 6 MB, phase A only
        for kc in range(8):
            nc.sync.dma_start(w3_sb[:, kc, ds(CH, 2 * CH)],
                              w3T[ts(kc, 128), ds(CH, 2 * CH)])
        for kc in range(8):
            nc.sync.dma_start(w3_sb[:, kc, ds(0, CH)], w3T[ts(kc, 128), ds(0, CH)])

        # ---- small resident tensors ----
        idx_sb = const.tile([128, NTB * 32], dt.int16)        # 0.25 MB
        nc.sync.dma_start(idx_sb[:], idx16[:, :])
        gb_sb = const.tile([128, NTB, 4 * NH], dt.float32)    # 1 MB
        nc.sync.dma_start(gb_sb[:], gb[:, :])
        s16_sb = const.tile([128, 16], dt.bfloat16)
        nc.sync.dma_start(s16_sb[:], s16d[:, :])
        qidx_sb = const.tile([128, NTB * 8], dt.int16)
        nc.sync.dma_start(qidx_sb[:], qidxd[:, :])
        ident_sb = const.tile([16, 16], dt.bfloat16)
        nc.sync.dma_start(ident_sb[:], identd[:, :])
        wo_sb = const.tile([128, 8, H], dt.bfloat16)          # 2 MB
        for chn in range(8):
            nc.sync.dma_start(wo_sb[:, chn, :], woT[ts(chn, 128), :])

        # ---- phase A: k/v first (AllGather can start early), then q ----
        kv_stores = []
        q_stores = []
        for st in range(NST):
            kvt_cur = None
            for pj in (1, 2):         # k then v
                ps = ps_big.tile([128, CH], dt.float32, tag="psb")
                for n in range(2):
                    for kc in range(8):
                        nc.tensor.matmul(
                            ps[:, ts(n, 512)],
                            xT_sb[:, kc, ts(st, 128)],
                            w3_sb[:, kc, ds(pj * CH + n * 512, 512)],
                            start=(kc == 0), stop=(kc == 7))
                if pj == 1:
                    kvt_cur = kv_pool.tile([128, 2, CH], dt.bfloat16, tag="kvt")
                    nc.scalar.copy(kvt_cur[:, 0, :], ps[:])
                else:
                    nc.scalar.copy(kvt_cur[:, 1, :], ps[:])
                    kv_stores.append(nc.sync.dma_start(
                        kv_loc[ts(st, 128), :],
                        kvt_cur[:].rearrange("p a b -> p (a b)")))
        cc_i = nc.gpsimd.collective_compute(
            "AllGather", mybir.AluOpType.bypass,
            replica_groups=[list(range(NCORES))],
            ins=[kv_loc[:, :]], outs=[kv_full[:, :]])
        for stn in kv_stores:
            add_dep_helper(cc_i.ins, stn.ins, sync=True, reason="cc after kv stores")
        cc_insts = [cc_i]
        for st in range(NST):         # q after all k/v (overlaps AllGather)
            ps = ps_big.tile([128, CH], dt.float32, tag="psb")
            for n in range(2):
                for kc in range(8):
                    nc.tensor.matmul(
                        ps[:, ts(n, 512)],
                        xT_sb[:, kc, ts(st, 128)],
                        w3_sb[:, kc, ds(n * 512, 512)],
                        start=(kc == 0), stop=(kc == 7))
            qt = kv_pool.tile([128, CH], dt.bfloat16, tag="qt")
            nc.scalar.copy(qt[:], ps[:])
            q_stores.append(nc.sync.dma_start(q_dram[ts(st, 128), :], qt[:]))
        wa.__exit__(None, None, None)
        gat = ctx.enter_context(tc.tile_pool(name="gat", bufs=3))
        big = ctx.enter_context(tc.tile_pool(name="big", bufs=2))
        small = ctx.enter_context(tc.tile_pool(name="small", bufs=4))
        atg_pool = ctx.enter_context(tc.tile_pool(name="atg", bufs=2))
        outp = ctx.enter_context(tc.tile_pool(name="outp", bufs=2))

        # ---- phase B: per-tile sparse attention ----
        atg_cur = None
        for t in range(NTB if mode != "proj" else 0):
            st, g16 = t // 8, t % 8
            # 1. gather k/v rows: [128, NCC, ROW]
            kvsel = gat.tile([128, NCC, ROW], dt.bfloat16, tag="kvsel")
            g = nc.gpsimd.dma_gather(
                out_ap=kvsel[:], in_ap=kv_full[:, :],
                idxs_ap=idx_sb[:, ds(t * 32, 32)],
                num_idxs=QT * K, num_idxs_reg=QT * K,
                elem_size=ROW, single_packet=False)
            for cc_i in cc_insts:
                add_dep_helper(g.ins, cc_i.ins, sync=True, reason="gather after cc")

            # 2. q replicated to 128 partitions (q[p%16]) via tiny row-gather
            qrep = small.tile([128, 1, CH], dt.bfloat16, tag="qrep")
            gq = nc.gpsimd.dma_gather(
                out_ap=qrep[:], in_ap=q_dram[:, :],
                idxs_ap=qidx_sb[:, ds(t * 8, 8)],
                num_idxs=128, num_idxs_reg=128,
                elem_size=CH, single_packet=False)
            add_dep_helper(gq.ins, q_stores[st].ins, sync=True,
                           reason="qrep gather after q store")

            # 3. t1 = qrep (bcast over chunks) * k_sel
            t1 = big.tile([128, NCC, CH], dt.bfloat16, tag="t1")
            k_ap = kvsel[:, :, 0:CH]
            k_ap2, q_ap2 = bass.broadcast_tensor_aps(k_ap, qrep[:, 0:1, :])
            nc.vector.tensor_mul(t1[:], k_ap2, q_ap2)

            # 4. logits[p, (cc,h)] = sum_d t1 -- pure-TT halving tree (2x mode;
            # tensor_reduce only has a 1x uop)
            th = small.tile([128, 4 * NH, 32], dt.bfloat16, tag="th")
            t1v = t1[:].rearrange("p c (h d) -> p (c h) d", d=D)
            nc.vector.tensor_add(th[:], t1v[:, :, 0:32], t1v[:, :, 32:64])
            nc.vector.tensor_add(th[:, :, 0:16], th[:, :, 0:16], th[:, :, 16:32])
            nc.vector.tensor_add(th[:, :, 0:8], th[:, :, 0:8], th[:, :, 8:16])
            nc.vector.tensor_add(th[:, :, 0:4], th[:, :, 0:4], th[:, :, 4:8])
            nc.vector.tensor_add(th[:, :, 0:2], th[:, :, 0:2], th[:, :, 2:4])
            lgt = small.tile([128, 4 * NH], dt.float32, tag="lgt")
            lgtv = lgt[:].rearrange("p (g o) -> p g o", o=1)
            nc.vector.tensor_add(lgtv, th[:, :, 0:1], th[:, :, 1:2])
            nc.vector.tensor_add(lgt[:], lgt[:], gb_sb[:, t, :])

            # 5. e2 = exp(logits), written pair-expanded for the 2x bcast muls
            e2 = small.tile([128, NCC, NH, 2], dt.bfloat16, tag="e2")
            lgt4 = lgt[:].rearrange("p (c h o) -> p c h o", c=NCC, o=1)
            e2a, lgt4b = bass.broadcast_tensor_aps(e2[:], lgt4)
            nc.scalar.activation(e2a, lgt4b, EXP)

            # 6. denominator: den[q, (h, w)] = sum_{b,cc} e  via S16 matmul
            # (both pair lanes carried through -> 1/den comes out already
            # pair-expanded); runs in parallel with the W/AV path below
            psd = ps_sm.tile([16, 2 * NH], dt.float32, tag="pss")
            for cc in range(NCC):
                nc.tensor.matmul(psd[:],
                                 s16_sb[:],
                                 e2[:, cc, :, :].rearrange("p h w -> p (h w)"),
                                 start=(cc == 0), stop=(cc == NCC - 1))
            r16 = small.tile([16, 2 * NH], dt.float32, tag="r16")
            nc.vector.reciprocal(r16[:], psd[:])
            r2 = small.tile([16, NH, 2], dt.bfloat16, tag="r2")
            nc.scalar.copy(r2[:].rearrange("p h w -> p (h w)"), r16[:])

            # 7. W = v_sel * e (bcast over d, pair-expanded so DVE hits 2x;
            # uses the unnormalized e so this never waits on the recip path)
            W = big.tile([128, NCC, CH], dt.bfloat16, tag="W")
            v_ap2, e_ap2 = bass.broadcast_tensor_aps(
                kvsel[:, :, CH:ROW].rearrange("p c (h dd w) -> p c h dd w", dd=32, w=2),
                e2[:].rearrange("p c h (dd w) -> p c h dd w", dd=1, w=2))
            nc.vector.tensor_mul(
                W[:].rearrange("p c (h dd w) -> p c h dd w", dd=32, w=2),
                v_ap2, e_ap2)

            # 8. A[q, hd] = sum_{b,cc} W  via S16 matmul (PSUM accumulate)
            psA = ps_big.tile([16, CH], dt.float32, tag="psb")
            for n in range(2):
                for cc in range(NCC):
                    nc.tensor.matmul(psA[:, ts(n, 512)], s16_sb[:],
                                     W[:, cc, ts(n, 512)],
                                     start=(cc == 0), stop=(cc == NCC - 1))
            A_raw = small.tile([16, CH], dt.bfloat16, tag="A_raw")
            nc.scalar.copy(A_raw[:], psA[:])

            # 9. normalize: A = A_raw * (1/den) (bcast over d, pair-expanded)
            A_sb = small.tile([16, CH], dt.bfloat16, tag="A_sb")
            a_in, r_in = bass.broadcast_tensor_aps(
                A_raw[:].rearrange("p (h dd w) -> p h dd w", dd=32, w=2),
                r2[:].rearrange("p h (dd w) -> p h dd w", dd=1, w=2))
            nc.vector.tensor_mul(
                A_sb[:].rearrange("p (h dd w) -> p h dd w", dd=32, w=2),
                a_in, r_in)

            # 10. A^T chunks via PE transpose -> group buffer [128, 8, 128]
            if g16 == 0:
                atg_cur = atg_pool.tile([128, 8, 128], dt.bfloat16, tag="atg")
            psT = ps_sm.tile([128, 8, QT], dt.bfloat16, tag="pss")
            for chk in range(8):
                nc.tensor.transpose(psT[:, chk, :], A_sb[:, ts(chk, 128)],
                                    ident_sb[:])
            nc.scalar.copy(atg_cur[:, :, ds(QT * g16, QT)], psT[:])

            # 11. o-proj per group of 8 tiles (128 query rows)
            if g16 == 7:
                psP = ps_big.tile([128, H], dt.float32, tag="psb")
                for n in range(2):
                    for chk in range(8):
                        nc.tensor.matmul(psP[:, ts(n, 512)], atg_cur[:, chk, :],
                                         wo_sb[:, chk, ts(n, 512)],
                                         start=(chk == 0), stop=(chk == 7))
                ot = outp.tile([128, H], dt.float32, tag="ot")
                nc.scalar.copy(ot[:], psP[:])
                nc.sync.dma_start(outd[ts(st, 128), :], ot[:])

    nc.compile()
    return nc


def prep_inputs(x, idx, valid, geo_bias, Wq, Wk, Wv, Wo, bo):
    """Host-side shard prep. Returns (in_maps, bo_f32)."""
    x = np.asarray(x)
    idx = np.asarray(idx)
    geo_bias = np.asarray(geo_bias)
    Wq, Wk, Wv, Wo = (np.asarray(w) for w in (Wq, Wk, Wv, Wo))
    bo = np.asarray(bo, dtype=np.float32)

    x2 = x.reshape(S, H)
    scale = np.float32(1.0 / np.sqrt(D))
    w3T = np.ascontiguousarray(
        np.concatenate([(Wq * scale).T, Wk.T, Wv.T], axis=1).astype(BF16))
    woT = np.ascontiguousarray(Wo.T.astype(BF16))
    s16 = np.zeros((128, 16), dtype=BF16)
    s16[np.arange(128), np.arange(128) % 16] = 1
    ident = np.eye(16, dtype=BF16)
    # qrep gather: tile t, pos p -> q row t*16 + p%16
    qidx = np.empty((16, NTB * 8), dtype=np.int16)
    for t in range(NTB):
        lin = (t * QT + np.arange(128) % 16).astype(np.int16)
        qidx[:, t * 8:(t + 1) * 8] = lin.reshape(8, 16).T
    qidx = np.ascontiguousarray(np.tile(qidx, (8, 1)))

    in_maps = []
    for c in range(NCORES):
        rb = c * SC
        xTc = np.ascontiguousarray(x2[rb:rb + SC].T.astype(BF16))

        # gather indices: tile t, pos = j*16 + q -> idx[rb + t*16 + q, j]
        idxc = np.empty((16, NTB * 32), dtype=np.int16)
        for t in range(NTB):
            blk = idx[rb + t * QT: rb + (t + 1) * QT, :]      # [16 q, 32 j]
            lin = blk.T.reshape(-1)                            # pos = j*16+q
            idxc[:, t * 32:(t + 1) * 32] = lin.reshape(32, 16).T.astype(np.int16)
        idxc = np.ascontiguousarray(np.tile(idxc, (8, 1)))

        # geo bias: gb[p=(b,qq), t, cc*16+h] = geo_bias[h, rb+t*16+qq, cc*8+b]
        gg = geo_bias[:, rb:rb + SC, :]                        # [h, 512, j]
        g2 = gg.reshape(NH, NTB, QT, NCC, 8)                   # [h, t, qq, cc, b]
        gbt = g2.transpose(4, 2, 1, 3, 0).reshape(128, NTB * 4 * NH)
        gbt = np.ascontiguousarray(gbt, dtype=np.float32)

        in_maps.append({
            "xT": xTc,
            "w3T": w3T,
            "woT": woT,
            "gb": gbt,
            "idx16": idxc,
            "s16": s16,
            "qidx16": qidx,
            "ident": ident,
        })
    return in_maps, bo


def kernel(x, idx, valid, geo_bias, Wq, Wk, Wv, Wo, bo):
    global _nc_cache
    from concourse.bass_utils import run_bass_kernel_spmd

    if _nc_cache is None:
        _nc_cache = build_nc()
    nc = _nc_cache

    in_maps, bo_f32 = prep_inputs(x, idx, valid, geo_bias, Wq, Wk, Wv, Wo, bo)
    res = run_bass_kernel_spmd(nc, in_maps, core_ids=list(range(NCORES)),
                               trace=bool(int(os.environ.get("KTRACE", "0"))))
    out = np.concatenate([r["out"] for r in res.results], axis=0)
    out = out + bo_f32[None, :]
    if res.exec_time_ns is not None:
        kernel.last_exec_time_ns = res.exec_time_ns
    kernel.last_results = res
    return out.reshape(1, S, H).astype(np.float32)


# revision 11
# speedup vs baseline: 1.1902x; 1.0930x over previous
"""Sparse-attention Trainium2 kernel (8 NeuronCores, sequence-parallel).

Problem (hardcoded): B=1, S=4096, H=1024, NH=16, D=64, K=32.

Sharding: fully sequence-parallel. Core c owns query rows [512c, 512c+512).
It computes q/k/v for its own rows against the FULL weight matrices (1/8 of
the total FLOPs, no communication), publishes its k|v rows via an
8-way AllGather (bf16; triggered as soon as the kv stores land, well before
the q projection finishes), then dma_gathers per-query k/v rows for
ALL 16 heads at once (4 KB/descriptor), computes the sparse attention for its
rows and the o-projection. Host concatenates row slices + bo.

Per-tile layout (16 queries/tile, 32 tiles/core): gather position
pos = j*16 + q -> SBUF partition p = 16*(j%8) + q, chunk cc = j//8, so
partition p holds query tb+p%16 and slot block b = p//16 (slot j = 8cc+b).
Cross-partition sums over the 8 slot blocks (softmax denominator, AV
accumulation) are TensorE matmuls against a static 0/1 selection matrix
S16[p, m] = (p%16 == m). q is replicated to 128 partitions by a tiny
128-descriptor dma_gather from q in DRAM.

DVE diet (the attention phase is Vector-bound): the d-reduction for the
logits is a pure tensor_tensor halving tree (tensor_reduce only has a 1x
uop; TT adds hit the 2x bf16 mode), exp writes the pair-expanded e2 tile
directly on ScalarE, and the softmax normalization is applied to e (one
[128, 4, 16, 2] mul against a PE-broadcast 1/den) instead of to the
[16, 1024] AV output, so the PSUM evacuation is a plain ScalarE copy.
Broadcast multiplies pair-expand the scalar operand so the VectorE runs
in its 2x bf16 mode; kvsel gathers use single_packet=False (large
single-packet SWDGE gathers hard-fault). Attention-side elementwise runs
on VectorE only; GpSimd does only early-bound work (gathers) -- putting
softmax-dependent ops on its in-order queue stalls future tiles' gathers.
"""

import os
from contextlib import ExitStack

import numpy as np
import ml_dtypes

S, H, NH, D, K = 4096, 1024, 16, 64, 32
NCORES = 8
SC = S // NCORES            # 512 rows per core
QT = 16                     # queries per attention tile
NTB = SC // QT              # 32 attention tiles per core
NST = SC // 128             # 4 projection s-tiles per core
CH = NH * D                 # 1024 kv channels per tensor
ROW = 2 * CH                # 2048 bf16 elems per kv row (4 KB)
NCC = K // 8                # 4 slot chunks per tile
BF16 = ml_dtypes.bfloat16

_nc_cache = None


def build_nc(mode="full"):
    import concourse.bass as bass
    import concourse.mybir as mybir
    import concourse.tile as tile
    from concourse import bacc
    from concourse.tile_rust import add_dep_helper
    from concourse.bass import ts, ds

    dt = mybir.dt
    nc = bacc.Bacc("TRN2", target_bir_lowering=False, debug=False,
                   num_devices=NCORES)

    xT = nc.dram_tensor("xT", [H, SC], dt.bfloat16, kind="ExternalInput")
    w3T = nc.dram_tensor("w3T", [H, 3 * CH], dt.bfloat16, kind="ExternalInput")
    woT = nc.dram_tensor("woT", [CH, H], dt.bfloat16, kind="ExternalInput")
    gb = nc.dram_tensor("gb", [128, NTB * 4 * NH], dt.float32, kind="ExternalInput")
    idx16 = nc.dram_tensor("idx16", [128, NTB * (QT * K // 16)], dt.int16,
                           kind="ExternalInput")
    s16d = nc.dram_tensor("s16", [128, 16], dt.bfloat16, kind="ExternalInput")
    qidxd = nc.dram_tensor("qidx16", [128, NTB * 8], dt.int16, kind="ExternalInput")
    identd = nc.dram_tensor("ident", [16, 16], dt.bfloat16, kind="ExternalInput")
    outd = nc.dram_tensor("out", [SC, H], dt.float32, kind="ExternalOutput")
    kv_loc = nc.dram_tensor("kv_loc", [SC, ROW], dt.bfloat16, kind="Internal")
    q_dram = nc.dram_tensor("q_dram", [SC, CH], dt.bfloat16, kind="Internal")
    kv_full = nc.dram_tensor("kv_full", [S, ROW], dt.bfloat16, kind="Internal",
                             addr_space="Shared")

    EXP = mybir.ActivationFunctionType.Exp
    COPY = mybir.ActivationFunctionType.Copy

    with ExitStack() as ctx:
        tc = ctx.enter_context(tile.TileContext(nc))
        const = ctx.enter_context(tc.tile_pool(name="const", bufs=1))

        kv_pool = ctx.enter_context(tc.tile_pool(name="kvout", bufs=2))
        ps_big = ctx.enter_context(tc.tile_pool(name="ps_big", bufs=3, space="PSUM"))
        ps_sm = ctx.enter_context(tc.tile_pool(name="ps_sm", bufs=2, space="PSUM"))

        # ---- phase-A weights first: the kv matmuls gate the collective ----
        wa = tc.tile_pool(name="wa", bufs=1)
        wap = wa.__enter__()
        xT_sb = wap.tile([128, 8, SC], dt.bfloat16)           # 1 MB, phase A only
        for kc in range(8):
            nc.sync.dma_start(xT_sb[:, kc, :], xT[ts(kc, 128), :])
        w3_sb = wap.tile([128, 8, 3 * CH], dt.bfloat16)       # 6 MB, phase A only
        for kc in range(8):
            nc.sync.dma_start(w3_sb[:, kc, ds(CH, 2 * CH)],
                              w3T[ts(kc, 128), ds(CH, 2 * CH)])
        for kc in range(8):
            nc.sync.dma_start(w3_sb[:, kc, ds(0, CH)], w3T[ts(kc, 128), ds(0, CH)])

        # ---- small resident tensors ----
        idx_sb = const.tile([128, NTB * 32], dt.int16)        # 0.25 MB
        nc.sync.dma_start(idx_sb[:], idx16[:, :])
        gb_sb = const.tile([128, NTB, 4 * NH], dt.float32)    # 1 MB
        nc.sync.dma_start(gb_sb[:], gb[:, :])
        s16_sb = const.tile([128, 16], dt.bfloat16)
        nc.sync.dma_start(s16_sb[:], s16d[:, :])
        qidx_sb = const.tile([128, NTB * 8], dt.int16)
        nc.sync.dma_start(qidx_sb[:], qidxd[:, :])
        ident_sb = const.tile([16, 16], dt.bfloat16)
        nc.sync.dma_start(ident_sb[:], identd[:, :])
        wo_sb = const.tile([128, 8, H], dt.bfloat16)          # 2 MB
        for chn in range(8):
            nc.sync.dma_start(wo_sb[:, chn, :], woT[ts(chn, 128), :])

        # ---- phase A: k/v first (AllGather can start early), then q ----
        kv_stores = []
        q_stores = []
        for st in range(NST):
            kvt_cur = None
            for pj in (1, 2):         # k then v
                ps = ps_big.tile([128, CH], dt.float32, tag="psb")
                for n in range(2):
                    for kc in range(8):
                        nc.tensor.matmul(
                            ps[:, ts(n, 512)],
                            xT_sb[:, kc, ts(st, 128)],
                            w3_sb[:, kc, ds(pj * CH + n * 512, 512)],
                            start=(kc == 0), stop=(kc == 7))
                if pj == 1:
                    kvt_cur = kv_pool.tile([128, 2, CH], dt.bfloat16, tag="kvt")
                    nc.scalar.copy(kvt_cur[:, 0, :], ps[:])
                else:
                    nc.scalar.copy(kvt_cur[:, 1, :], ps[:])
                    kv_stores.append(nc.sync.dma_start(
                        kv_loc[ts(st, 128), :],
                        kvt_cur[:].rearrange("p a b -> p (a b)")))
        cc_i = nc.gpsimd.collective_compute(
            "AllGather", mybir.AluOpType.bypass,
            replica_groups=[list(range(NCORES))],
            ins=[kv_loc[:, :]], outs=[kv_full[:, :]])
        for stn in kv_stores:
            add_dep_helper(cc_i.ins, stn.ins, sync=True, reason="cc after kv stores")
        cc_insts = [cc_i]
        for st in range(NST):         # q after all k/v (overlaps AllGather)
            ps = ps_big.tile([128, CH], dt.float32, tag="psb")
            for n in range(2):
                for kc in range(8):
                    nc.tensor.matmul(
                        ps[:, ts(n, 512)],
                        xT_sb[:, kc, ts(st, 128)],
                        w3_sb[:, kc, ds(n * 512, 512)],
                        start=(kc == 0), stop=(kc == 7))
            qt = kv_pool.tile([128, CH], dt.bfloat16, tag="qt")
            nc.scalar.copy(qt[:], ps[:])
            q_stores.append(nc.sync.dma_start(q_dram[ts(st, 128), :], qt[:]))
        wa.__exit__(None, None, None)
        gat = ctx.enter_context(tc.tile_pool(name="gat", bufs=4))
        qst_pool = ctx.enter_context(tc.tile_pool(name="qst", bufs=2))
        big = ctx.enter_context(tc.tile_pool(name="big", bufs=2))
        small = ctx.enter_context(tc.tile_pool(name="small", bufs=4))
        atg_pool = ctx.enter_context(tc.tile_pool(name="atg", bufs=2))
        outp = ctx.enter_context(tc.tile_pool(name="outp", bufs=1))

        # ---- phase B: per-tile sparse attention ----
        def qst_gather(st):
            """One batched gather for all 8 tiles of an s-tile: q rows
            replicated to [128, 8, CH] (tile tt at [:, tt%8, :])."""
            qst = qst_pool.tile([128, 8, CH], dt.bfloat16, tag="qst")
            gq = nc.gpsimd.dma_gather(
                out_ap=qst[:], in_ap=q_dram[:, :],
                idxs_ap=qidx_sb[:, ds(st * 64, 64)],
                num_idxs=8 * 128, num_idxs_reg=8 * 128,
                elem_size=CH, single_packet=False)
            add_dep_helper(gq.ins, q_stores[st].ins, sync=True,
                           reason="qst gather after q store")
            return qst

        atg_cur = None
        qst_cur = qst_gather(0) if mode != "proj" else None
        qst_next = None
        for t in range(NTB if mode != "proj" else 0):
            st, g16 = t // 8, t % 8
            # 1. gather k/v rows: [128, NCC, ROW]
            kvsel = gat.tile([128, NCC, ROW], dt.bfloat16, tag="kvsel")
            g = nc.gpsimd.dma_gather(
                out_ap=kvsel[:], in_ap=kv_full[:, :],
                idxs_ap=idx_sb[:, ds(t * 32, 32)],
                num_idxs=QT * K, num_idxs_reg=QT * K,
                elem_size=ROW, single_packet=False)
            for cc_i in cc_insts:
                add_dep_helper(g.ins, cc_i.ins, sync=True, reason="gather after cc")

            # 2. prefetch next s-tile's replicated q
            if g16 == 0 and st + 1 < NST:
                qst_next = qst_gather(st + 1)

            # 3. t1 = qrep (bcast over chunks) * k_sel
            qrep = qst_cur[:, g16:g16 + 1, :]
            t1 = big.tile([128, NCC, CH], dt.bfloat16, tag="t1")
            k_ap = kvsel[:, :, 0:CH]
            k_ap2, q_ap2 = bass.broadcast_tensor_aps(k_ap, qrep)
            nc.vector.tensor_mul(t1[:], k_ap2, q_ap2)

            # 4. logits[p, (cc,h)] = sum_d t1 -- pure-TT halving tree (2x
            # mode; tensor_reduce only has a 1x uop), in place inside t1
            t1v = t1[:].rearrange("p c (h d) -> p (c h) d", d=D)
            nc.vector.tensor_add(t1v[:, :, 0:32], t1v[:, :, 0:32], t1v[:, :, 32:64])
            nc.vector.tensor_add(t1v[:, :, 0:16], t1v[:, :, 0:16], t1v[:, :, 16:32])
            nc.vector.tensor_add(t1v[:, :, 0:8], t1v[:, :, 0:8], t1v[:, :, 8:16])
            nc.vector.tensor_add(t1v[:, :, 0:4], t1v[:, :, 0:4], t1v[:, :, 4:8])
            nc.vector.tensor_add(t1v[:, :, 0:2], t1v[:, :, 0:2], t1v[:, :, 2:4])
            lgt = small.tile([128, 4 * NH], dt.float32, tag="lgt")
            lgtv = lgt[:].rearrange("p (g o) -> p g o", o=1)
            nc.vector.tensor_add(lgtv, t1v[:, :, 0:1], t1v[:, :, 1:2])
            nc.vector.tensor_add(lgt[:], lgt[:], gb_sb[:, t, :])

            # 5. e2 = exp(logits), written pair-expanded for the 2x bcast muls
            e2 = small.tile([128, NCC, NH, 2], dt.bfloat16, tag="e2")
            lgt4 = lgt[:].rearrange("p (c h o) -> p c h o", c=NCC, o=1)
            e2a, lgt4b = bass.broadcast_tensor_aps(e2[:], lgt4)
            nc.scalar.activation(e2a, lgt4b, EXP)

            # 6. denominator: den[q, (h, w)] = sum_{b,cc} e  via S16 matmul
            # (both pair lanes carried through -> 1/den comes out already
            # pair-expanded); runs in parallel with the W/AV path below
            psd = ps_sm.tile([16, 2 * NH], dt.float32, tag="pss")
            for cc in range(NCC):
                nc.tensor.matmul(psd[:],
                                 s16_sb[:],
                                 e2[:, cc, :, :].rearrange("p h w -> p (h w)"),
                                 start=(cc == 0), stop=(cc == NCC - 1))
            r16 = small.tile([16, 2 * NH], dt.float32, tag="r16")
            nc.vector.reciprocal(r16[:], psd[:])
            r2 = small.tile([16, NH, 2], dt.bfloat16, tag="r2")
            nc.scalar.copy(r2[:].rearrange("p h w -> p (h w)"), r16[:])

            # 7. W = v_sel * e (bcast over d, pair-expanded so DVE hits 2x;
            # uses the unnormalized e so this never waits on the recip path)
            W = big.tile([128, NCC, CH], dt.bfloat16, tag="W")
            v_ap2, e_ap2 = bass.broadcast_tensor_aps(
                kvsel[:, :, CH:ROW].rearrange("p c (h dd w) -> p c h dd w", dd=32, w=2),
                e2[:].rearrange("p c h (dd w) -> p c h dd w", dd=1, w=2))
            nc.vector.tensor_mul(
                W[:].rearrange("p c (h dd w) -> p c h dd w", dd=32, w=2),
                v_ap2, e_ap2)

            # 8. A[q, hd] = sum_{b,cc} W  via S16 matmul (PSUM accumulate)
            psA = ps_big.tile([16, CH], dt.float32, tag="psb")
            for n in range(2):
                for cc in range(NCC):
                    nc.tensor.matmul(psA[:, ts(n, 512)], s16_sb[:],
                                     W[:, cc, ts(n, 512)],
                                     start=(cc == 0), stop=(cc == NCC - 1))
            A_raw = small.tile([16, CH], dt.bfloat16, tag="A_raw")
            nc.scalar.copy(A_raw[:], psA[:])

            # 9. normalize: A = A_raw * (1/den) (bcast over d, pair-expanded)
            A_sb = small.tile([16, CH], dt.bfloat16, tag="A_sb")
            a_in, r_in = bass.broadcast_tensor_aps(
                A_raw[:].rearrange("p (h dd w) -> p h dd w", dd=32, w=2),
                r2[:].rearrange("p h (dd w) -> p h dd w", dd=1, w=2))
            nc.vector.tensor_mul(
                A_sb[:].rearrange("p (h dd w) -> p h dd w", dd=32, w=2),
                a_in, r_in)

            # 10. A^T chunks via PE transpose -> group buffer [128, 8, 128]
            if g16 == 0:
                atg_cur = atg_pool.tile([128, 8, 128], dt.bfloat16, tag="atg")
            psT = ps_sm.tile([128, 8, QT], dt.bfloat16, tag="pss")
            for chk in range(8):
                nc.tensor.transpose(psT[:, chk, :], A_sb[:, ts(chk, 128)],
                                    ident_sb[:])
            nc.scalar.copy(atg_cur[:, :, ds(QT * g16, QT)], psT[:])

            # 11. o-proj per group of 8 tiles (128 query rows)
            if g16 == 7:
                psP = ps_big.tile([128, H], dt.float32, tag="psb")
                for n in range(2):
                    for chk in range(8):
                        nc.tensor.matmul(psP[:, ts(n, 512)], atg_cur[:, chk, :],
                                         wo_sb[:, chk, ts(n, 512)],
                                         start=(chk == 0), stop=(chk == 7))
                ot = outp.tile([128, H], dt.float32, tag="ot")
                nc.scalar.copy(ot[:], psP[:])
                nc.sync.dma_start(outd[ts(st, 128), :], ot[:])
                qst_cur = qst_next

    nc.compile()
    return nc


def prep_inputs(x, idx, valid, geo_bias, Wq, Wk, Wv, Wo, bo):
    """Host-side shard prep. Returns (in_maps, bo_f32)."""
    x = np.asarray(x)
    idx = np.asarray(idx)
    geo_bias = np.asarray(geo_bias)
    Wq, Wk, Wv, Wo = (np.asarray(w) for w in (Wq, Wk, Wv, Wo))
    bo = np.asarray(bo, dtype=np.float32)

    x2 = x.reshape(S, H)
    scale = np.float32(1.0 / np.sqrt(D))
    w3T = np.ascontiguousarray(
        np.concatenate([(Wq * scale).T, Wk.T, Wv.T], axis=1).astype(BF16))
    woT = np.ascontiguousarray(Wo.T.astype(BF16))
    s16 = np.zeros((128, 16), dtype=BF16)
    s16[np.arange(128), np.arange(128) % 16] = 1
    ident = np.eye(16, dtype=BF16)
    # qrep gather: tile t, pos p -> q row t*16 + p%16
    qidx = np.empty((16, NTB * 8), dtype=np.int16)
    for t in range(NTB):
        lin = (t * QT + np.arange(128) % 16).astype(np.int16)
        qidx[:, t * 8:(t + 1) * 8] = lin.reshape(8, 16).T
    qidx = np.ascontiguousarray(np.tile(qidx, (8, 1)))

    in_maps = []
    for c in range(NCORES):
        rb = c * SC
        xTc = np.ascontiguousarray(x2[rb:rb + SC].T.astype(BF16))

        # gather indices: tile t, pos = j*16 + q -> idx[rb + t*16 + q, j]
        idxc = np.empty((16, NTB * 32), dtype=np.int16)
        for t in range(NTB):
            blk = idx[rb + t * QT: rb + (t + 1) * QT, :]      # [16 q, 32 j]
            lin = blk.T.reshape(-1)                            # pos = j*16+q
            idxc[:, t * 32:(t + 1) * 32] = lin.reshape(32, 16).T.astype(np.int16)
        idxc = np.ascontiguousarray(np.tile(idxc, (8, 1)))

        # geo bias: gb[p=(b,qq), t, cc*16+h] = geo_bias[h, rb+t*16+qq, cc*8+b]
        gg = geo_bias[:, rb:rb + SC, :]                        # [h, 512, j]
        g2 = gg.reshape(NH, NTB, QT, NCC, 8)                   # [h, t, qq, cc, b]
        gbt = g2.transpose(4, 2, 1, 3, 0).reshape(128, NTB * 4 * NH)
        gbt = np.ascontiguousarray(gbt, dtype=np.float32)

        in_maps.append({
            "xT": xTc,
            "w3T": w3T,
            "woT": woT,
            "gb": gbt,
            "idx16": idxc,
            "s16": s16,
            "qidx16": qidx,
            "ident": ident,
        })
    return in_maps, bo


def kernel(x, idx, valid, geo_bias, Wq, Wk, Wv, Wo, bo):
    global _nc_cache
    from concourse.bass_utils import run_bass_kernel_spmd

    if _nc_cache is None:
        _nc_cache = build_nc()
    nc = _nc_cache

    in_maps, bo_f32 = prep_inputs(x, idx, valid, geo_bias, Wq, Wk, Wv, Wo, bo)
    res = run_bass_kernel_spmd(nc, in_maps, core_ids=list(range(NCORES)),
                               trace=bool(int(os.environ.get("KTRACE", "0"))))
    out = np.concatenate([r["out"] for r in res.results], axis=0)
    out = out + bo_f32[None, :]
    if res.exec_time_ns is not None:
        kernel.last_exec_time_ns = res.exec_time_ns
    kernel.last_results = res
    return out.reshape(1, S, H).astype(np.float32)
